# revision 28
# baseline (speedup 1.0000x reference)
"""ConvSelfAttention Trainium2 kernel.

Reference computation (B=4, C=512, N=2048, H=8 heads, D=64):
    qkv = w_qkv @ x          (pointwise conv == matmul over channels)
    per head: sim = (q*D^-.5)^T k ; attn = softmax(sim, axis=j)
    out = attn @ v^T ; y = w_out @ out_heads + b_out

Sharding: 8 cores = 4 batches x 2 head-groups (4 heads each). Each core
computes its batch's x-projections restricted to its 4 heads, runs
attention, and produces a partial output projection y_part[c, n]
(sum over its heads' hd columns of w_out). Host sums the two partials
per batch and adds the bias.

On-chip layout notes:
  - Matmuls take bf16 inputs (fp32 PSUM accumulation).
  - Attention is computed transposed: S^T[j, i] = k^T q in [128, 1024]
    PSUM chunks (one j-block, both heads of the pair). exp runs wide
    from PSUM: most chunks on ScalarE (true Exp), a fraction on the DVE
    via the Schraudolph bit trick (t = S*128*log2(e) + 128*(127+c) cast
    to int16 == the bf16 bit pattern of e^S; rms err ~2% on those
    slots, diluted by the exact slots and the 2e-2 budget).
  - PV runs transposed: out[i-block, 65] += pt[j, i-block]^T [v^T | 1],
    with pt as the stationary matmul operand. Cost-wise this moves only
    65 columns per accumulation pass (vs 512 in the direct layout), and
    the softmax denominator l lands per-partition (column 64), so
    normalization is one reciprocal + one scalar_tensor_tensor per head
    half -- no partition-broadcast matmul.
  - The normalized o^T[i, (iblk, hd)] tile is block-transposed back to
    o[hd, i] on the DMA xbar (dma_start_transpose), costing no PE or
    DVE time, then the output projection consumes it as in the direct
    layout.
  - Heads are processed in pairs living in SBUF partitions 0-63/64-127
    so the K=64 S^T matmuls auto-derive tile_position (0,0)/(64,0).
"""

import numpy as np
import ml_dtypes

B, C, N = 4, 512, 2048
H, D = 8, 64
HID = H * D
SCALE = D ** -0.5
NCORES = 8
HPC = 4          # heads per core
NT = 4           # i-tiles of 512
KT = 4           # k-tiles of 128 over C
JB = 16          # j-blocks of 128
SLOT = 512

# Schraudolph constants: int16(S*A + BIAS) == bf16 bits of e^S (approx).
LOG2E = 1.4426950408889634
SCH_A = float(np.float32(128.0 * LOG2E))
SCH_B = float(np.float32(128.0 * (127.0 - 0.0430)))
# j-blocks whose exp runs on the DVE via Schraudolph (rest: ScalarE Exp)
DVE_JBS = frozenset({1, 3, 5, 8, 10, 12, 14})

bf16 = ml_dtypes.bfloat16

_PROG = None


def _build_program(reps=1):
    import concourse.mybir as mybir
    import concourse.tile as tile
    from concourse import bacc

    fp32 = mybir.dt.float32
    bfl = mybir.dt.bfloat16
    i16 = mybir.dt.int16
    Exp = mybir.ActivationFunctionType.Exp
    Alu = mybir.AluOpType

    nc = bacc.Bacc("TRN2", target_bir_lowering=False, debug=False)

    x_d = nc.dram_tensor("x", [C, N], bfl, kind="ExternalInput")
    wqk_d = nc.dram_tensor("wqk", [C, 512], bfl, kind="ExternalInput")
    wv_d = nc.dram_tensor("wv", [C, 256], bfl, kind="ExternalInput")
    wo_d = nc.dram_tensor("wo", [256, C], bfl, kind="ExternalInput")
    y_d = nc.dram_tensor("y", [C, N], bfl, kind="ExternalOutput")

    import contextlib

    with tile.TileContext(nc) as tc:
        loop_cm = tc.For_i(0, reps, 1) if reps > 1 else contextlib.nullcontext()
        with (
            loop_cm,
            tc.tile_pool(name="big", bufs=1) as bigp,
            tc.tile_pool(name="pt", bufs=1) as ptp,
            tc.tile_pool(name="ov", bufs=1) as ovp,
            tc.tile_pool(name="spsum", bufs=3, space="PSUM") as sp,
            tc.tile_pool(name="wpsum", bufs=2, space="PSUM") as wp,
        ):
            # ---- input loads, coalesced: one DMA per logical tensor chunk
            # (fewer HWDGE slots), nt-major x so the first matmuls can start
            # after ~0.5MB instead of the full 3MB.
            wqk_all = bigp.tile([128, KT, 512], bfl, tag="wqk", name="wqk")
            x_all = bigp.tile([128, KT, N], bfl, tag="x", name="x")
            wv_all = bigp.tile([128, KT, 256], bfl, tag="wv", name="wv")
            wo_all = bigp.tile([128, 2, 512], bfl, tag="wo", name="wo")
            # the first psum group (qkproj mt0/nt0) consumes wqk-kt + x-kt
            # sequentially; interleave those DMAs so each kt's matmul can
            # start as soon as its slice lands.
            for kt in range(KT):
                nc.sync.dma_start(
                    wqk_all[:, kt, :], wqk_d[kt * 128:(kt + 1) * 128, :]
                )
                nc.sync.dma_start(
                    x_all[:, kt, 0:SLOT],
                    x_d[kt * 128:(kt + 1) * 128, 0:SLOT],
                )
            nc.sync.dma_start(
                wv_all[:],
                wv_d[:, :].rearrange("(kt p) c -> p kt c", p=128),
            )
            for nt in range(1, NT):
                nc.sync.dma_start(
                    x_all[:, :, nt * SLOT:(nt + 1) * SLOT],
                    x_d[:, nt * SLOT:(nt + 1) * SLOT].rearrange(
                        "(kt p) n -> p kt n", p=128
                    ),
                )
            nc.sync.dma_start(
                wo_all[:],
                wo_d[:, :].rearrange("(kt p) c -> p kt c", p=128),
            )
            wqk_sb = [wqk_all[:, kt, :] for kt in range(KT)]
            x_sb = [x_all[:, kt, :] for kt in range(KT)]
            wv_sb = [wv_all[:, kt, :] for kt in range(KT)]
            wo_sb = [wo_all[:, kt, :] for kt in range(2)]

            # ---- QK projection -> q_sb[hp], k_sb[hp] (2 heads stacked)
            # host column order: q-hp0 | k-hp0 | q-hp1 | k-hp1.
            q_sb = [bigp.tile([128, N], bfl, tag=f"q{hp}", name=f"q{hp}") for hp in range(2)]
            k_sb = [bigp.tile([128, N], bfl, tag=f"k{hp}", name=f"k{hp}") for hp in range(2)]
            dest = [q_sb[0], k_sb[0], q_sb[1], k_sb[1]]

            def emit_qk_unit(mt, nt):
                ps = wp.tile([128, SLOT], fp32, tag="w", name="wps")
                for kt in range(KT):
                    nc.tensor.matmul(
                        ps[:],
                        wqk_sb[kt][:, mt * 128:(mt + 1) * 128],
                        x_sb[kt][:, nt * SLOT:(nt + 1) * SLOT],
                        start=(kt == 0),
                        stop=(kt == KT - 1),
                    )
                nc.vector.tensor_copy(
                    dest[mt][:, nt * SLOT:(nt + 1) * SLOT], ps[:]
                )

            # ---- v^T projection, augmented with a ones column per head
            # vt_sb[jb] : [128 (j), HPC, 65] ; [:, h, 0:64] = v^T, [:, h, 64] = 1
            vt_sb = [
                bigp.tile([128, HPC, 65], bfl, tag=f"vt{jb}", name=f"vt{jb}")
                for jb in range(JB)
            ]

            def emit_vt_unit(jb):
                t = vt_sb[jb]
                nc.gpsimd.memset(t[:, :, 64:65], 1.0)  # Pool engine: idle anyway
                ps = wp.tile([128, SLOT], fp32, tag="w", name="wps")
                for kt in range(KT):
                    nc.tensor.matmul(
                        ps[:, 0:256],
                        x_sb[kt][:, jb * 128:(jb + 1) * 128],
                        wv_sb[kt][:],
                        start=(kt == 0),
                        stop=(kt == KT - 1),
                    )
                # ScalarE copy: the DVE is loaded with qk copies + exp early on
                nc.scalar.copy(
                    t[:, :, 0:64],
                    ps[:, 0:256].rearrange("p (h d) -> p h d", h=HPC),
                )

            # ---- attention: per (it, hp) group, 16 chunks of [128, 1024]
            # (chunk jb = S^T for both heads of the pair at j-block jb).
            oh_sb = [bigp.tile([128, N], bfl, tag=f"oh{hp}", name=f"oh{hp}") for hp in range(2)]
            groups = [(it, hp) for it in range(NT) for hp in range(2)]

            def emit_st_chunk(it, hp, jb, dve_set=DVE_JBS):
                """S^T matmuls + exp for one chunk. Returns (kind, pt_tile)."""
                st = sp.tile([128, 2 * SLOT], fp32, tag="st", name="st")
                for hh in range(2):
                    lo, hi = hh * 64, hh * 64 + 64
                    nc.tensor.matmul(
                        st[:, hh * SLOT:(hh + 1) * SLOT],
                        k_sb[hp][lo:hi, jb * 128:(jb + 1) * 128],
                        q_sb[hp][lo:hi, it * SLOT:(it + 1) * SLOT],
                    )
                if jb in dve_set:
                    pt = ptp.tile([128, 2 * SLOT], i16, tag="pti", bufs=14, name="pti")
                    nc.vector.tensor_scalar(
                        pt[:], st[:], SCH_A, SCH_B, op0=Alu.mult, op1=Alu.add
                    )
                    return ("i", pt)
                pt = ptp.tile([128, 2 * SLOT], bfl, tag="ptb", bufs=22, name="ptb")
                nc.scalar.activation(pt[:], st[:], Exp)
                return ("b", pt)

            def emit_pv_half(hp, hh, pts, pv_pair):
                """PV^T for one head of a group. PSUM accumulation groups must
                be contiguous start->stop within a bank, so jb is innermost."""
                for ib in range(4):
                    for jb in range(JB):
                        kind, pt = pts[jb]
                        ptv = (
                            pt[:].bitcast(mybir.dt.bfloat16)
                            if kind == "i" else pt[:]
                        )
                        nc.tensor.matmul(
                            pv_pair[hh][:, ib * 128:ib * 128 + 65],
                            ptv[0:128, hh * SLOT + ib * 128:hh * SLOT + (ib + 1) * 128],
                            vt_sb[jb][:, hp * 2 + hh, :],
                            start=(jb == 0),
                            stop=(jb == JB - 1),
                        )

            def emit_norm(hh, pv_pair, o2):
                # o2[i, ib, hh, d] = pv/l for one head (the xbar transpose
                # needs a full 128-partition destination, so both heads share
                # one o2 tile and a single transpose).
                rl = ovp.tile([128, 4], fp32, tag="rl", bufs=4, name="rl")
                nc.vector.reciprocal(rl[:], pv_pair[hh][:, 64::128])
                nc.vector.scalar_tensor_tensor(
                    o2[:, :, hh, :],
                    pv_pair[hh][:].rearrange("p (a b) -> p a b", a=4)[:, :, 0:64],
                    1.0,
                    rl[:].unsqueeze(2).broadcast_to([128, 4, 64]),
                    op0=Alu.mult,
                    op1=Alu.mult,
                )

            def emit_outproj(it):
                for mt in range(4):
                    yp = wp.tile([128, SLOT], fp32, tag="w", name="yp")
                    for kt in range(2):
                        nc.tensor.matmul(
                            yp[:],
                            wo_sb[kt][:, mt * 128:(mt + 1) * 128],
                            oh_sb[kt][:, it * SLOT:(it + 1) * SLOT],
                            start=(kt == 0),
                            stop=(kt == 1),
                        )
                    ys = ovp.tile([128, SLOT], bfl, tag="ys", bufs=4, name="ys")
                    # split the psum->sbuf copies between DVE and ScalarE
                    if mt % 2 == 0:
                        nc.vector.tensor_copy(ys[:], yp[:])
                    else:
                        nc.scalar.copy(ys[:], yp[:])
                    nc.sync.dma_start(
                        y_d[mt * 128:(mt + 1) * 128, it * SLOT:(it + 1) * SLOT],
                        ys[:],
                    )

            # ---- software pipeline: produce S^T/exp chunks for group g+1,
            # then consume group g (PV burst + norm + transpose + outproj).
            def consume(it, hp, pts):
                pv_pair = [
                    wp.tile([128, SLOT], fp32, tag="w", name="pv0"),
                    wp.tile([128, SLOT], fp32, tag="w", name="pv1"),
                ]
                o2 = ovp.tile([128, 4, 2, 64], bfl, tag="o2", bufs=2, name="o2")
                for hh in range(2):
                    emit_pv_half(hp, hh, pts, pv_pair)
                    emit_norm(hh, pv_pair, o2)
                nc.sync.dma_start_transpose(
                    oh_sb[hp][:, it * SLOT:(it + 1) * SLOT].rearrange(
                        "p (a b) -> p a b", a=4
                    ),
                    o2[:].rearrange("p a h d -> p (a h d)"),
                )
                if hp == 1:
                    emit_outproj(it)

            emit_qk_unit(0, 0)
            emit_qk_unit(1, 0)
            emit_qk_unit(2, 0)
            emit_qk_unit(3, 0)

            # remaining projection work, dripped between group-0 chunk
            # emissions so the PE always has queued work while the exp
            # engines catch up and the x DMA stream lands. Each unit is
            # placed after the chunk whose x/nt dependencies precede it;
            # qk(1, nt) must precede chunk jb = 4*nt (S^T needs k columns).
            unit_sched = {
                2: [(emit_vt_unit, (jb,)) for jb in range(4)],
                3: [(emit_qk_unit, (1, 1))],
                4: [(emit_qk_unit, (2, 1)), (emit_qk_unit, (3, 1)),
                    (emit_qk_unit, (0, 1))]
                   + [(emit_vt_unit, (jb,)) for jb in range(4, 8)],
                7: [(emit_qk_unit, (1, 2))],
                8: [(emit_qk_unit, (2, 2)), (emit_qk_unit, (3, 2)),
                    (emit_qk_unit, (0, 2))]
                   + [(emit_vt_unit, (jb,)) for jb in range(8, 12)],
                11: [(emit_qk_unit, (1, 3))],
                12: [(emit_qk_unit, (2, 3)), (emit_qk_unit, (3, 3)),
                     (emit_qk_unit, (0, 3))]
                    + [(emit_vt_unit, (jb,)) for jb in range(12, JB)],
            }

            # Lighter DVE exp share for the last two groups: the group-g norm
            # (DVE) gates group-g+1's PV via the PSUM ring, and at the tail
            # the DVE queue must drain fast.
            dve_sets = {6: frozenset({3, 5, 8, 11, 13, 15}),
                        7: frozenset({3, 6, 9, 12, 15})}

            pending = None
            for gi, (it, hp) in enumerate(groups):
                pts = []
                for jb in range(JB):
                    pts.append(emit_st_chunk(it, hp, jb, dve_sets.get(gi, DVE_JBS)))
                    if gi == 0:
                        for fn, args in unit_sched.get(jb, ()):
                            fn(*args)
                if pending is not None:
                    consume(*pending)
                pending = (it, hp, pts)
            consume(*pending)

    nc.compile()
    return nc


def _get_program():
    global _PROG
    if _PROG is None:
        _PROG = _build_program()
    return _PROG


def _make_in_maps(x, w_qkv, w_out):
    xs = x.astype(bf16)
    in_maps = []
    for core in range(NCORES):
        b, g = divmod(core, 2)
        r0 = 256 * g
        wq = w_qkv[r0:r0 + 256, :] * SCALE
        wk = w_qkv[512 + r0:512 + r0 + 256, :]
        wv = w_qkv[1024 + r0:1024 + r0 + 256, :]
        wqkT = np.ascontiguousarray(
            np.concatenate(
                [wq[0:128], wk[0:128], wq[128:256], wk[128:256]], axis=0
            ).T.astype(bf16)
        )
        wvT = np.ascontiguousarray(wv.T.astype(bf16))
        woT = np.ascontiguousarray(w_out[:, r0:r0 + 256].T.astype(bf16))
        in_maps.append({
            "x": np.ascontiguousarray(xs[b]),
            "wqk": wqkT,
            "wv": wvT,
            "wo": woT,
        })
    return in_maps


def _gather(results, b_out):
    parts = [results[c]["y"].astype(np.float32) for c in range(NCORES)]
    y = np.stack([parts[2 * b] + parts[2 * b + 1] for b in range(B)])
    y += b_out[None, :, None]
    return y.astype(np.float32)


def kernel(x, w_qkv, w_out, b_out):
    from concourse.bass_utils import run_bass_kernel_spmd

    nc = _get_program()

    x = np.asarray(x, dtype=np.float32)
    w_qkv = np.asarray(w_qkv, dtype=np.float32)
    w_out = np.asarray(w_out, dtype=np.float32)
    b_out = np.asarray(b_out, dtype=np.float32)

    in_maps = _make_in_maps(x, w_qkv, w_out)
    res = run_bass_kernel_spmd(nc, in_maps, core_ids=list(range(NCORES)))
    return _gather(res.results, b_out)


# revision 30
# speedup vs baseline: 1.0216x; 1.0216x over previous
"""ConvSelfAttention Trainium2 kernel.

Reference computation (B=4, C=512, N=2048, H=8 heads, D=64):
    qkv = w_qkv @ x          (pointwise conv == matmul over channels)
    per head: sim = (q*D^-.5)^T k ; attn = softmax(sim, axis=j)
    out = attn @ v^T ; y = w_out @ out_heads + b_out

Sharding: 8 cores = 4 batches x 2 head-groups (4 heads each). Each core
computes its batch's x-projections restricted to its 4 heads, runs
attention, and produces a partial output projection y_part[c, n]
(sum over its heads' hd columns of w_out). Host sums the two partials
per batch and adds the bias.

On-chip layout notes:
  - Matmuls take bf16 inputs (fp32 PSUM accumulation).
  - Attention is computed transposed: S^T[j, i] = k^T q in [128, 1024]
    PSUM chunks (one j-block, both heads of the pair). exp runs wide
    from PSUM: most chunks on ScalarE (true Exp), a fraction on the DVE
    via the Schraudolph bit trick (t = S*128*log2(e) + 128*(127+c) cast
    to int16 == the bf16 bit pattern of e^S; rms err ~2% on those
    slots, diluted by the exact slots and the 2e-2 budget).
  - PV runs transposed: out[i-block, 65] += pt[j, i-block]^T [v^T | 1],
    with pt as the stationary matmul operand. Cost-wise this moves only
    65 columns per accumulation pass (vs 512 in the direct layout), and
    the softmax denominator l lands per-partition (column 64), so
    normalization is one reciprocal + one scalar_tensor_tensor per head
    half -- no partition-broadcast matmul.
  - The normalized o^T[i, (iblk, hd)] tile is block-transposed back to
    o[hd, i] on the DMA xbar (dma_start_transpose), costing no PE or
    DVE time, then the output projection consumes it as in the direct
    layout.
  - Heads are processed in pairs living in SBUF partitions 0-63/64-127
    so the K=64 S^T matmuls auto-derive tile_position (0,0)/(64,0).
"""

import numpy as np
import ml_dtypes

B, C, N = 4, 512, 2048
H, D = 8, 64
HID = H * D
SCALE = D ** -0.5
NCORES = 8
HPC = 4          # heads per core
NT = 4           # i-tiles of 512
KT = 4           # k-tiles of 128 over C
JB = 16          # j-blocks of 128
SLOT = 512

# Schraudolph constants: int16(S*A + BIAS) == bf16 bits of e^S (approx).
LOG2E = 1.4426950408889634
SCH_A = float(np.float32(128.0 * LOG2E))
SCH_B = float(np.float32(128.0 * (127.0 - 0.0430)))
# j-blocks whose exp runs on the DVE via Schraudolph (rest: ScalarE Exp)
DVE_JBS = frozenset({1, 3, 5, 8, 10, 12, 14})

bf16 = ml_dtypes.bfloat16

_PROG = None


def _build_program(reps=1):
    import concourse.mybir as mybir
    import concourse.tile as tile
    from concourse import bacc

    fp32 = mybir.dt.float32
    bfl = mybir.dt.bfloat16
    i16 = mybir.dt.int16
    Exp = mybir.ActivationFunctionType.Exp
    Alu = mybir.AluOpType

    nc = bacc.Bacc("TRN2", target_bir_lowering=False, debug=False)

    x_d = nc.dram_tensor("x", [C, N], bfl, kind="ExternalInput")
    wqk_d = nc.dram_tensor("wqk", [C, 512], bfl, kind="ExternalInput")
    wv_d = nc.dram_tensor("wv", [C, 256], bfl, kind="ExternalInput")
    wo_d = nc.dram_tensor("wo", [256, C], bfl, kind="ExternalInput")
    y_d = nc.dram_tensor("y", [C, N], bfl, kind="ExternalOutput")

    import contextlib

    with tile.TileContext(nc) as tc:
        loop_cm = tc.For_i(0, reps, 1) if reps > 1 else contextlib.nullcontext()
        with (
            loop_cm,
            tc.tile_pool(name="big", bufs=1) as bigp,
            tc.tile_pool(name="pt", bufs=1) as ptp,
            tc.tile_pool(name="ov", bufs=1) as ovp,
            tc.tile_pool(name="spsum", bufs=3, space="PSUM") as sp,
            tc.tile_pool(name="wpsum", bufs=2, space="PSUM") as wp,
        ):
            # ---- input loads, coalesced: one DMA per logical tensor chunk
            # (fewer HWDGE slots), nt-major x so the first matmuls can start
            # after ~0.5MB instead of the full 3MB.
            wqk_all = bigp.tile([128, KT, 512], bfl, tag="wqk", name="wqk")
            x_all = bigp.tile([128, KT, N], bfl, tag="x", name="x")
            wv_all = bigp.tile([128, KT, 256], bfl, tag="wv", name="wv")
            wo_all = bigp.tile([128, 2, 512], bfl, tag="wo", name="wo")
            # first matmul (kt0 of qkproj mt0/nt0) only needs wqk-kt0 + x-kt0:
            # issue those first so the PE starts ~1us earlier.
            nc.sync.dma_start(wqk_all[:, 0, :], wqk_d[0:128, :])
            nc.sync.dma_start(x_all[:, 0, 0:SLOT], x_d[0:128, 0:SLOT])
            nc.sync.dma_start(
                wqk_all[:, 1:4, :],
                wqk_d[128:512, :].rearrange("(kt p) c -> p kt c", p=128),
            )
            nc.sync.dma_start(
                x_all[:, 1:4, 0:SLOT],
                x_d[128:512, 0:SLOT].rearrange("(kt p) n -> p kt n", p=128),
            )
            nc.sync.dma_start(
                wv_all[:],
                wv_d[:, :].rearrange("(kt p) c -> p kt c", p=128),
            )
            for nt in range(1, NT):
                nc.sync.dma_start(
                    x_all[:, :, nt * SLOT:(nt + 1) * SLOT],
                    x_d[:, nt * SLOT:(nt + 1) * SLOT].rearrange(
                        "(kt p) n -> p kt n", p=128
                    ),
                )
            nc.sync.dma_start(
                wo_all[:],
                wo_d[:, :].rearrange("(kt p) c -> p kt c", p=128),
            )
            wqk_sb = [wqk_all[:, kt, :] for kt in range(KT)]
            x_sb = [x_all[:, kt, :] for kt in range(KT)]
            wv_sb = [wv_all[:, kt, :] for kt in range(KT)]
            wo_sb = [wo_all[:, kt, :] for kt in range(2)]

            # ---- QK projection -> q_sb[hp], k_sb[hp] (2 heads stacked)
            # host column order: q-hp0 | k-hp0 | q-hp1 | k-hp1.
            q_sb = [bigp.tile([128, N], bfl, tag=f"q{hp}", name=f"q{hp}") for hp in range(2)]
            k_sb = [bigp.tile([128, N], bfl, tag=f"k{hp}", name=f"k{hp}") for hp in range(2)]
            dest = [q_sb[0], k_sb[0], q_sb[1], k_sb[1]]

            def emit_qk_unit(mt, nt):
                ps = wp.tile([128, SLOT], fp32, tag="w", name="wps")
                for kt in range(KT):
                    nc.tensor.matmul(
                        ps[:],
                        wqk_sb[kt][:, mt * 128:(mt + 1) * 128],
                        x_sb[kt][:, nt * SLOT:(nt + 1) * SLOT],
                        start=(kt == 0),
                        stop=(kt == KT - 1),
                    )
                nc.vector.tensor_copy(
                    dest[mt][:, nt * SLOT:(nt + 1) * SLOT], ps[:]
                )

            # ---- v^T projection, augmented with a ones column per head
            # vt_sb[jb] : [128 (j), HPC, 65] ; [:, h, 0:64] = v^T, [:, h, 64] = 1
            vt_sb = [
                bigp.tile([128, HPC, 65], bfl, tag=f"vt{jb}", name=f"vt{jb}")
                for jb in range(JB)
            ]

            def emit_vt_unit(jb):
                t = vt_sb[jb]
                nc.gpsimd.memset(t[:, :, 64:65], 1.0)  # Pool engine: idle anyway
                ps = wp.tile([128, SLOT], fp32, tag="w", name="wps")
                for kt in range(KT):
                    nc.tensor.matmul(
                        ps[:, 0:256],
                        x_sb[kt][:, jb * 128:(jb + 1) * 128],
                        wv_sb[kt][:],
                        start=(kt == 0),
                        stop=(kt == KT - 1),
                    )
                # ScalarE copy: the DVE is loaded with qk copies + exp early on
                nc.scalar.copy(
                    t[:, :, 0:64],
                    ps[:, 0:256].rearrange("p (h d) -> p h d", h=HPC),
                )

            # ---- attention: per (it, hp) group, 16 chunks of [128, 1024]
            # (chunk jb = S^T for both heads of the pair at j-block jb).
            oh_sb = [bigp.tile([128, N], bfl, tag=f"oh{hp}", name=f"oh{hp}") for hp in range(2)]
            groups = [(it, hp) for it in range(NT) for hp in range(2)]

            def emit_st_chunk(it, hp, jb, dve_set=DVE_JBS):
                """S^T matmuls + exp for one chunk. Returns (kind, pt_tile)."""
                st = sp.tile([128, 2 * SLOT], fp32, tag="st", name="st")
                for hh in range(2):
                    lo, hi = hh * 64, hh * 64 + 64
                    nc.tensor.matmul(
                        st[:, hh * SLOT:(hh + 1) * SLOT],
                        k_sb[hp][lo:hi, jb * 128:(jb + 1) * 128],
                        q_sb[hp][lo:hi, it * SLOT:(it + 1) * SLOT],
                    )
                if jb in dve_set:
                    pt = ptp.tile([128, 2 * SLOT], i16, tag="pti", bufs=14, name="pti")
                    nc.vector.tensor_scalar(
                        pt[:], st[:], SCH_A, SCH_B, op0=Alu.mult, op1=Alu.add
                    )
                    return ("i", pt)
                pt = ptp.tile([128, 2 * SLOT], bfl, tag="ptb", bufs=22, name="ptb")
                nc.scalar.activation(pt[:], st[:], Exp)
                return ("b", pt)

            def emit_pv_half(hp, hh, pts, pv_pair):
                """PV^T for one head of a group. PSUM accumulation groups must
                be contiguous start->stop within a bank, so jb is innermost."""
                for ib in range(4):
                    for jb in range(JB):
                        kind, pt = pts[jb]
                        ptv = (
                            pt[:].bitcast(mybir.dt.bfloat16)
                            if kind == "i" else pt[:]
                        )
                        nc.tensor.matmul(
                            pv_pair[hh][:, ib * 128:ib * 128 + 65],
                            ptv[0:128, hh * SLOT + ib * 128:hh * SLOT + (ib + 1) * 128],
                            vt_sb[jb][:, hp * 2 + hh, :],
                            start=(jb == 0),
                            stop=(jb == JB - 1),
                        )

            def emit_norm(hh, pv_pair, o2):
                # o2[i, ib, hh, d] = pv/l for one head (the xbar transpose
                # needs a full 128-partition destination, so both heads share
                # one o2 tile and a single transpose).
                rl = ovp.tile([128, 4], fp32, tag="rl", bufs=4, name="rl")
                nc.vector.reciprocal(rl[:], pv_pair[hh][:, 64::128])
                nc.vector.scalar_tensor_tensor(
                    o2[:, :, hh, :],
                    pv_pair[hh][:].rearrange("p (a b) -> p a b", a=4)[:, :, 0:64],
                    1.0,
                    rl[:].unsqueeze(2).broadcast_to([128, 4, 64]),
                    op0=Alu.mult,
                    op1=Alu.mult,
                )

            def emit_outproj(it):
                for mt in range(4):
                    yp = wp.tile([128, SLOT], fp32, tag="w", name="yp")
                    for kt in range(2):
                        nc.tensor.matmul(
                            yp[:],
                            wo_sb[kt][:, mt * 128:(mt + 1) * 128],
                            oh_sb[kt][:, it * SLOT:(it + 1) * SLOT],
                            start=(kt == 0),
                            stop=(kt == 1),
                        )
                    ys = ovp.tile([128, SLOT], bfl, tag="ys", bufs=4, name="ys")
                    # split the psum->sbuf copies between DVE and ScalarE
                    if mt % 2 == 0:
                        nc.vector.tensor_copy(ys[:], yp[:])
                    else:
                        nc.scalar.copy(ys[:], yp[:])
                    nc.sync.dma_start(
                        y_d[mt * 128:(mt + 1) * 128, it * SLOT:(it + 1) * SLOT],
                        ys[:],
                    )

            # ---- software pipeline: produce S^T/exp chunks for group g+1,
            # then consume group g (PV burst + norm + transpose + outproj).
            def consume(it, hp, pts):
                pv_pair = [
                    wp.tile([128, SLOT], fp32, tag="w", name="pv0"),
                    wp.tile([128, SLOT], fp32, tag="w", name="pv1"),
                ]
                o2 = ovp.tile([128, 4, 2, 64], bfl, tag="o2", bufs=2, name="o2")
                for hh in range(2):
                    emit_pv_half(hp, hh, pts, pv_pair)
                    emit_norm(hh, pv_pair, o2)
                nc.sync.dma_start_transpose(
                    oh_sb[hp][:, it * SLOT:(it + 1) * SLOT].rearrange(
                        "p (a b) -> p a b", a=4
                    ),
                    o2[:].rearrange("p a h d -> p (a h d)"),
                )
                if hp == 1:
                    emit_outproj(it)

            emit_qk_unit(0, 0)
            for nt in range(NT):
                emit_qk_unit(1, nt)

            # remaining projection work, interleaved between group-0 chunk
            # emissions so the PE always has queued work while the exp
            # engines catch up.
            units = []
            units += [(emit_vt_unit, (jb,)) for jb in range(4)]
            units += [(emit_qk_unit, (0, nt)) for nt in range(1, NT)]
            units += [(emit_qk_unit, (2, nt)) for nt in range(NT)]
            units += [(emit_qk_unit, (3, nt)) for nt in range(NT)]
            units += [(emit_vt_unit, (jb,)) for jb in range(4, JB)]
            ui = 0

            def drip_units(upto):
                nonlocal ui
                while ui < min(upto, len(units)):
                    fn, args = units[ui]
                    fn(*args)
                    ui += 1

            pending = None
            for gi, (it, hp) in enumerate(groups):
                pts = []
                for jb in range(JB):
                    pts.append(emit_st_chunk(it, hp, jb))
                    if gi == 0:
                        drip_units((jb + 1) * 2)
                if pending is not None:
                    consume(*pending)
                pending = (it, hp, pts)
            consume(*pending)

    nc.compile()
    return nc


def _get_program():
    global _PROG
    if _PROG is None:
        _PROG = _build_program()
    return _PROG


def _make_in_maps(x, w_qkv, w_out):
    xs = x.astype(bf16)
    in_maps = []
    for core in range(NCORES):
        b, g = divmod(core, 2)
        r0 = 256 * g
        wq = w_qkv[r0:r0 + 256, :] * SCALE
        wk = w_qkv[512 + r0:512 + r0 + 256, :]
        wv = w_qkv[1024 + r0:1024 + r0 + 256, :]
        wqkT = np.ascontiguousarray(
            np.concatenate(
                [wq[0:128], wk[0:128], wq[128:256], wk[128:256]], axis=0
            ).T.astype(bf16)
        )
        wvT = np.ascontiguousarray(wv.T.astype(bf16))
        woT = np.ascontiguousarray(w_out[:, r0:r0 + 256].T.astype(bf16))
        in_maps.append({
            "x": np.ascontiguousarray(xs[b]),
            "wqk": wqkT,
            "wv": wvT,
            "wo": woT,
        })
    return in_maps


def _gather(results, b_out):
    parts = [results[c]["y"].astype(np.float32) for c in range(NCORES)]
    y = np.stack([parts[2 * b] + parts[2 * b + 1] for b in range(B)])
    y += b_out[None, :, None]
    return y.astype(np.float32)


def kernel(x, w_qkv, w_out, b_out):
    from concourse.bass_utils import run_bass_kernel_spmd

    nc = _get_program()

    x = np.asarray(x, dtype=np.float32)
    w_qkv = np.asarray(w_qkv, dtype=np.float32)
    w_out = np.asarray(w_out, dtype=np.float32)
    b_out = np.asarray(b_out, dtype=np.float32)

    in_maps = _make_in_maps(x, w_qkv, w_out)
    res = run_bass_kernel_spmd(nc, in_maps, core_ids=list(range(NCORES)))
    return _gather(res.results, b_out)


# revision 33
# speedup vs baseline: 1.0361x; 1.0142x over previous
"""ConvSelfAttention Trainium2 kernel.

Reference computation (B=4, C=512, N=2048, H=8 heads, D=64):
    qkv = w_qkv @ x          (pointwise conv == matmul over channels)
    per head: sim = (q*D^-.5)^T k ; attn = softmax(sim, axis=j)
    out = attn @ v^T ; y = w_out @ out_heads + b_out

Sharding: 8 cores = 4 batches x 2 head-groups (4 heads each). Each core
computes its batch's x-projections restricted to its 4 heads, runs
attention, and produces a partial output projection y_part[c, n]
(sum over its heads' hd columns of w_out). Host sums the two partials
per batch and adds the bias.

On-chip layout notes:
  - Matmuls take bf16 inputs (fp32 PSUM accumulation).
  - Attention is computed transposed: S^T[j, i] = k^T q in [128, 1024]
    PSUM chunks (one j-block, both heads of the pair). exp runs wide
    from PSUM: most chunks on ScalarE (true Exp), a fraction on the DVE
    via the Schraudolph bit trick (t = S*128*log2(e) + 128*(127+c) cast
    to int16 == the bf16 bit pattern of e^S; rms err ~2% on those
    slots, diluted by the exact slots and the 2e-2 budget).
  - PV runs transposed: out[i-block, 65] += pt[j, i-block]^T [v^T | 1],
    with pt as the stationary matmul operand. Cost-wise this moves only
    65 columns per accumulation pass (vs 512 in the direct layout), and
    the softmax denominator l lands per-partition (column 64), so
    normalization is one reciprocal + one scalar_tensor_tensor per head
    half -- no partition-broadcast matmul.
  - The normalized o^T[i, (iblk, hd)] tile is block-transposed back to
    o[hd, i] on the DMA xbar (dma_start_transpose), costing no PE or
    DVE time, then the output projection consumes it as in the direct
    layout.
  - Heads are processed in pairs living in SBUF partitions 0-63/64-127
    so the K=64 S^T matmuls auto-derive tile_position (0,0)/(64,0).
"""

import numpy as np
import ml_dtypes

B, C, N = 4, 512, 2048
H, D = 8, 64
HID = H * D
SCALE = D ** -0.5
NCORES = 8
HPC = 4          # heads per core
NT = 4           # i-tiles of 512
KT = 4           # k-tiles of 128 over C
JB = 16          # j-blocks of 128
SLOT = 512

# Schraudolph constants: int16(S*A + BIAS) == bf16 bits of e^S (approx).
LOG2E = 1.4426950408889634
SCH_A = float(np.float32(128.0 * LOG2E))
SCH_B = float(np.float32(128.0 * (127.0 - 0.0430)))
# j-blocks whose exp runs on the DVE via Schraudolph (rest: ScalarE Exp)
DVE_JBS = frozenset({1, 3, 5, 8, 10, 12, 14})

bf16 = ml_dtypes.bfloat16

_PROG = None


def _build_program(reps=1):
    import concourse.mybir as mybir
    import concourse.tile as tile
    from concourse import bacc
    from concourse.masks import make_identity

    fp32 = mybir.dt.float32
    bfl = mybir.dt.bfloat16
    i16 = mybir.dt.int16
    Exp = mybir.ActivationFunctionType.Exp
    Alu = mybir.AluOpType

    nc = bacc.Bacc("TRN2", target_bir_lowering=False, debug=False)

    x_d = nc.dram_tensor("x", [C, N], bfl, kind="ExternalInput")
    wqk_d = nc.dram_tensor("wqk", [C, 512], bfl, kind="ExternalInput")
    wv_d = nc.dram_tensor("wv", [C, 256], bfl, kind="ExternalInput")
    wo_d = nc.dram_tensor("wo", [256, C], bfl, kind="ExternalInput")
    y_d = nc.dram_tensor("y", [C, N], bfl, kind="ExternalOutput")

    import contextlib

    with tile.TileContext(nc) as tc:
        loop_cm = tc.For_i(0, reps, 1) if reps > 1 else contextlib.nullcontext()
        with (
            loop_cm,
            tc.tile_pool(name="big", bufs=1) as bigp,
            tc.tile_pool(name="pt", bufs=1) as ptp,
            tc.tile_pool(name="ov", bufs=1) as ovp,
            tc.tile_pool(name="spsum", bufs=3, space="PSUM") as sp,
            tc.tile_pool(name="wpsum", bufs=2, space="PSUM") as wp,
        ):
            # ---- input loads, coalesced: one DMA per logical tensor chunk
            # (fewer HWDGE slots), nt-major x so the first matmuls can start
            # after ~0.5MB instead of the full 3MB.
            ident = bigp.tile([128, 128], bfl, tag="ident", name="ident")
            make_identity(nc, ident[:])

            wqk_all = bigp.tile([128, KT, 512], bfl, tag="wqk", name="wqk")
            x_all = bigp.tile([128, KT, N], bfl, tag="x", name="x")
            wv_all = bigp.tile([128, KT, 256], bfl, tag="wv", name="wv")
            wo_all = bigp.tile([128, 2, 512], bfl, tag="wo", name="wo")
            # first matmul (kt0 of qkproj mt0/nt0) only needs wqk-kt0 + x-kt0:
            # issue those first so the PE starts ~1us earlier.
            nc.sync.dma_start(wqk_all[:, 0, :], wqk_d[0:128, :])
            nc.sync.dma_start(x_all[:, 0, 0:SLOT], x_d[0:128, 0:SLOT])
            nc.sync.dma_start(
                wqk_all[:, 1:4, :],
                wqk_d[128:512, :].rearrange("(kt p) c -> p kt c", p=128),
            )
            nc.sync.dma_start(
                x_all[:, 1:4, 0:SLOT],
                x_d[128:512, 0:SLOT].rearrange("(kt p) n -> p kt n", p=128),
            )
            nc.sync.dma_start(
                wv_all[:],
                wv_d[:, :].rearrange("(kt p) c -> p kt c", p=128),
            )
            for nt in range(1, NT):
                nc.sync.dma_start(
                    x_all[:, :, nt * SLOT:(nt + 1) * SLOT],
                    x_d[:, nt * SLOT:(nt + 1) * SLOT].rearrange(
                        "(kt p) n -> p kt n", p=128
                    ),
                )
            nc.sync.dma_start(
                wo_all[:],
                wo_d[:, :].rearrange("(kt p) c -> p kt c", p=128),
            )
            wqk_sb = [wqk_all[:, kt, :] for kt in range(KT)]
            x_sb = [x_all[:, kt, :] for kt in range(KT)]
            wv_sb = [wv_all[:, kt, :] for kt in range(KT)]
            wo_sb = [wo_all[:, kt, :] for kt in range(2)]

            # ---- QK projection -> q_sb[hp], k_sb[hp] (2 heads stacked)
            # host column order: q-hp0 | k-hp0 | q-hp1 | k-hp1.
            q_sb = [bigp.tile([128, N], bfl, tag=f"q{hp}", name=f"q{hp}") for hp in range(2)]
            k_sb = [bigp.tile([128, N], bfl, tag=f"k{hp}", name=f"k{hp}") for hp in range(2)]
            dest = [q_sb[0], k_sb[0], q_sb[1], k_sb[1]]

            def emit_qk_unit(mt, nt):
                ps = wp.tile([128, SLOT], fp32, tag="w", name="wps")
                for kt in range(KT):
                    nc.tensor.matmul(
                        ps[:],
                        wqk_sb[kt][:, mt * 128:(mt + 1) * 128],
                        x_sb[kt][:, nt * SLOT:(nt + 1) * SLOT],
                        start=(kt == 0),
                        stop=(kt == KT - 1),
                    )
                nc.vector.tensor_copy(
                    dest[mt][:, nt * SLOT:(nt + 1) * SLOT], ps[:]
                )

            # ---- v^T projection, augmented with a ones column per head
            # vt_sb[jb] : [128 (j), HPC, 65] ; [:, h, 0:64] = v^T, [:, h, 64] = 1
            vt_sb = [
                bigp.tile([128, HPC, 65], bfl, tag=f"vt{jb}", name=f"vt{jb}")
                for jb in range(JB)
            ]

            def emit_vt_unit(jb):
                t = vt_sb[jb]
                nc.gpsimd.memset(t[:, :, 64:65], 1.0)  # Pool engine: idle anyway
                ps = wp.tile([128, SLOT], fp32, tag="w", name="wps")
                for kt in range(KT):
                    nc.tensor.matmul(
                        ps[:, 0:256],
                        x_sb[kt][:, jb * 128:(jb + 1) * 128],
                        wv_sb[kt][:],
                        start=(kt == 0),
                        stop=(kt == KT - 1),
                    )
                # ScalarE copy: the DVE is loaded with qk copies + exp early on
                nc.scalar.copy(
                    t[:, :, 0:64],
                    ps[:, 0:256].rearrange("p (h d) -> p h d", h=HPC),
                )

            # ---- attention: per (it, hp) group, 16 chunks of [128, 1024]
            # (chunk jb = S^T for both heads of the pair at j-block jb).
            oh_sb = [bigp.tile([128, N], bfl, tag=f"oh{hp}", name=f"oh{hp}") for hp in range(2)]
            groups = [(it, hp) for it in range(NT) for hp in range(2)]

            def emit_st_chunk(it, hp, jb, dve_set=DVE_JBS):
                """S^T matmuls + exp for one chunk. Returns (kind, pt_tile)."""
                st = sp.tile([128, 2 * SLOT], fp32, tag="st", name="st")
                for hh in range(2):
                    lo, hi = hh * 64, hh * 64 + 64
                    nc.tensor.matmul(
                        st[:, hh * SLOT:(hh + 1) * SLOT],
                        k_sb[hp][lo:hi, jb * 128:(jb + 1) * 128],
                        q_sb[hp][lo:hi, it * SLOT:(it + 1) * SLOT],
                    )
                if jb in dve_set:
                    pt = ptp.tile([128, 2 * SLOT], i16, tag="pti", bufs=14, name="pti")
                    nc.vector.tensor_scalar(
                        pt[:], st[:], SCH_A, SCH_B, op0=Alu.mult, op1=Alu.add
                    )
                    return ("i", pt)
                pt = ptp.tile([128, 2 * SLOT], bfl, tag="ptb", bufs=22, name="ptb")
                nc.scalar.activation(pt[:], st[:], Exp)
                return ("b", pt)

            def emit_pv_half(hp, hh, pts, pv_pair):
                """PV^T for one head of a group. PSUM accumulation groups must
                be contiguous start->stop within a bank, so jb is innermost."""
                for ib in range(4):
                    for jb in range(JB):
                        kind, pt = pts[jb]
                        ptv = (
                            pt[:].bitcast(mybir.dt.bfloat16)
                            if kind == "i" else pt[:]
                        )
                        nc.tensor.matmul(
                            pv_pair[hh][:, ib * 128:ib * 128 + 65],
                            ptv[0:128, hh * SLOT + ib * 128:hh * SLOT + (ib + 1) * 128],
                            vt_sb[jb][:, hp * 2 + hh, :],
                            start=(jb == 0),
                            stop=(jb == JB - 1),
                        )

            def emit_norm(hh, pv_pair, o2):
                # o2[i, ib, hh, d] = pv/l for one head (the xbar transpose
                # needs a full 128-partition destination, so both heads share
                # one o2 tile and a single transpose).
                rl = ovp.tile([128, 4], fp32, tag="rl", bufs=4, name="rl")
                nc.vector.reciprocal(rl[:], pv_pair[hh][:, 64::128])
                nc.vector.scalar_tensor_tensor(
                    o2[:, :, hh, :],
                    pv_pair[hh][:].rearrange("p (a b) -> p a b", a=4)[:, :, 0:64],
                    1.0,
                    rl[:].unsqueeze(2).broadcast_to([128, 4, 64]),
                    op0=Alu.mult,
                    op1=Alu.mult,
                )

            def emit_outproj(it):
                for mt in range(4):
                    yp = wp.tile([128, SLOT], fp32, tag="w", name="yp")
                    for kt in range(2):
                        nc.tensor.matmul(
                            yp[:],
                            wo_sb[kt][:, mt * 128:(mt + 1) * 128],
                            oh_sb[kt][:, it * SLOT:(it + 1) * SLOT],
                            start=(kt == 0),
                            stop=(kt == 1),
                        )
                    ys = ovp.tile([128, SLOT], bfl, tag="ys", bufs=4, name="ys")
                    # split the psum->sbuf copies between DVE and ScalarE
                    if mt % 2 == 0:
                        nc.vector.tensor_copy(ys[:], yp[:])
                    else:
                        nc.scalar.copy(ys[:], yp[:])
                    nc.sync.dma_start(
                        y_d[mt * 128:(mt + 1) * 128, it * SLOT:(it + 1) * SLOT],
                        ys[:],
                    )

            # ---- software pipeline: produce S^T/exp chunks for group g+1,
            # then consume group g (PV burst + norm + transpose + outproj).
            def consume(it, hp, pts):
                pv_pair = [
                    wp.tile([128, SLOT], fp32, tag="w", name="pv0"),
                    wp.tile([128, SLOT], fp32, tag="w", name="pv1"),
                ]
                o2 = ovp.tile([128, 4, 2, 64], bfl, tag="o2", bufs=2, name="o2")
                for hh in range(2):
                    emit_pv_half(hp, hh, pts, pv_pair)
                    emit_norm(hh, pv_pair, o2)
                if (it, hp) == (NT - 1, 1):
                    # final group: the DMA-xbar transpose's fixed plumbing
                    # (~2.6us) would sit on the critical path; the PE is idle
                    # here, so transpose on it instead.
                    tr = wp.tile([128, SLOT], bfl, tag="w", name="tr")
                    for ib in range(4):
                        nc.tensor.transpose(
                            tr[:, ib * 128:(ib + 1) * 128],
                            o2[:, ib, :, :].rearrange("p h d -> p (h d)"),
                            ident[:],
                        )
                    nc.vector.tensor_copy(
                        oh_sb[hp][:, it * SLOT:(it + 1) * SLOT], tr[:]
                    )
                else:
                    nc.sync.dma_start_transpose(
                        oh_sb[hp][:, it * SLOT:(it + 1) * SLOT].rearrange(
                            "p (a b) -> p a b", a=4
                        ),
                        o2[:].rearrange("p a h d -> p (a h d)"),
                    )
                if hp == 1:
                    emit_outproj(it)

            emit_qk_unit(0, 0)
            for nt in range(NT):
                emit_qk_unit(1, nt)

            # remaining projection work, interleaved between group-0 chunk
            # emissions so the PE always has queued work while the exp
            # engines catch up.
            units = []
            units += [(emit_vt_unit, (jb,)) for jb in range(4)]
            units += [(emit_qk_unit, (0, nt)) for nt in range(1, NT)]
            units += [(emit_qk_unit, (2, nt)) for nt in range(NT)]
            units += [(emit_qk_unit, (3, nt)) for nt in range(NT)]
            units += [(emit_vt_unit, (jb,)) for jb in range(4, JB)]
            ui = 0

            def drip_units(upto):
                nonlocal ui
                while ui < min(upto, len(units)):
                    fn, args = units[ui]
                    fn(*args)
                    ui += 1

            pending = None
            for gi, (it, hp) in enumerate(groups):
                pts = []
                for jb in range(JB):
                    pts.append(emit_st_chunk(it, hp, jb))
                    if gi == 0:
                        drip_units((jb + 1) * 2)
                if pending is not None:
                    consume(*pending)
                pending = (it, hp, pts)
            consume(*pending)

    nc.compile()
    return nc


def _get_program():
    global _PROG
    if _PROG is None:
        _PROG = _build_program()
    return _PROG


def _make_in_maps(x, w_qkv, w_out):
    xs = x.astype(bf16)
    in_maps = []
    for core in range(NCORES):
        b, g = divmod(core, 2)
        r0 = 256 * g
        wq = w_qkv[r0:r0 + 256, :] * SCALE
        wk = w_qkv[512 + r0:512 + r0 + 256, :]
        wv = w_qkv[1024 + r0:1024 + r0 + 256, :]
        wqkT = np.ascontiguousarray(
            np.concatenate(
                [wq[0:128], wk[0:128], wq[128:256], wk[128:256]], axis=0
            ).T.astype(bf16)
        )
        wvT = np.ascontiguousarray(wv.T.astype(bf16))
        woT = np.ascontiguousarray(w_out[:, r0:r0 + 256].T.astype(bf16))
        in_maps.append({
            "x": np.ascontiguousarray(xs[b]),
            "wqk": wqkT,
            "wv": wvT,
            "wo": woT,
        })
    return in_maps


def _gather(results, b_out):
    parts = [results[c]["y"].astype(np.float32) for c in range(NCORES)]
    y = np.stack([parts[2 * b] + parts[2 * b + 1] for b in range(B)])
    y += b_out[None, :, None]
    return y.astype(np.float32)


def kernel(x, w_qkv, w_out, b_out):
    from concourse.bass_utils import run_bass_kernel_spmd

    nc = _get_program()

    x = np.asarray(x, dtype=np.float32)
    w_qkv = np.asarray(w_qkv, dtype=np.float32)
    w_out = np.asarray(w_out, dtype=np.float32)
    b_out = np.asarray(b_out, dtype=np.float32)

    in_maps = _make_in_maps(x, w_qkv, w_out)
    res = run_bass_kernel_spmd(nc, in_maps, core_ids=list(range(NCORES)))
    return _gather(res.results, b_out)
